# revision 10
# baseline (speedup 1.0000x reference)
"""Trainium2 Bass kernel for the Koopman operator nn.Module.

Per-channel tiny MLPs (4 real channels, 6 complex-conjugate pairs, H=64,
2 hidden layers) over 65536 flattened batch elements, then a block-diagonal
Koopman update.  Pure data parallel over 8 NeuronCores (8192 elements each).

v2 strategy (weight-stationary, fp16 matmul path, transpose-free input):
  - host uploads z in three layouts: elem-major [128, 64, 16] for the final
    combine, strip-packed channel-major z1/z2 (at partition bases 0/32/64/96)
    for the on-device |z|^2, and fp16 zr rows DMA'd straight into the MLP
    input tile -- no input transposes on the tensor engine
  - all matmuls fp16 (1 cycle/row) with fp32 PSUM accumulation
  - 4 quarters x (L0 / hid0 / hid1 / final phases across all 5 pair-blocks):
    consecutive matmuls share stationary weights and the program order lets
    pair j's ReLUs drain while pairs j+1.. stream, keeping the PE p-state
    ramped at 2.4 GHz
  - ReLUs round-robin over DVE / Act / GpSimd (three engines)
  - final-layer outputs go back to elem-major via 4 PE transposes per chunk;
    exp/sin/combine run as a handful of big batched ops at the end
    (sin(x+pi/2) for cos); activation table loads ~2 for the whole kernel
"""

import numpy as np

NR, NCC, L, H = 4, 6, 2, 64
B, S, C = 32, 2048, 16
NCORES = 8
F_CORE = B * S // NCORES        # 8192 elements per core
CHUNK = 512                     # elements per matmul chunk (one PSUM bank)
NCH = F_CORE // CHUNK           # 16 chunks
NQ = 4                          # quarters
KPQ = NCH // NQ                 # 4 chunks per quarter
NSTRIP = 2                      # xcat partition bands at bases 0 and 64
STRIP = F_CORE // NSTRIP        # 4096 elements per strip
CPS = STRIP // CHUNK            # 8 chunks per strip

HALF_PI = float(np.pi / 2)

# wcat column layout (fp16): w0(5x128) | wm0(5x128) | wm1(5x128) | wl(5x128)
# | ident(128)
W0_OFF, WM0_OFF, WM1_OFF, WL_OFF, ID_OFF = 0, 640, 1280, 1920, 2560
WCAT_COLS = 2688
# bcat column layout (fp32): b0(5) | bm0(5) | bm1(5) | bias128(1) | pi/2(1)
BCAT_COLS = 17

_cached_nc = None


def _build():
    import concourse.tile as tile
    from concourse import bacc, mybir

    f32 = mybir.dt.float32
    f16 = mybir.dt.float16
    RELU = mybir.ActivationFunctionType.Relu
    IDENT = mybir.ActivationFunctionType.Identity
    SQUARE = mybir.ActivationFunctionType.Square
    EXP = mybir.ActivationFunctionType.Exp
    SIN = mybir.ActivationFunctionType.Sin
    ADD = mybir.AluOpType.add
    MAX = mybir.AluOpType.max

    nc = bacc.Bacc("TRN2", target_bir_lowering=False, debug=False,
                   num_devices=NCORES)

    zin_d = nc.dram_tensor("zin", [128, NCH, KPQ, C], f32,
                           kind="ExternalInput").ap()
    z12_d = nc.dram_tensor("z12", [2, 128, STRIP], f32,
                           kind="ExternalInput").ap()
    xzr_d = nc.dram_tensor("xzr", [NSTRIP, 4, STRIP], f16,
                           kind="ExternalInput").ap()
    wcat_d = nc.dram_tensor("wcat", [128, WCAT_COLS], f16,
                            kind="ExternalInput").ap()
    bcat_d = nc.dram_tensor("bcat", [128, BCAT_COLS], f32,
                            kind="ExternalInput").ap()
    out_d = nc.dram_tensor("out", [128, NCH, KPQ, C], f32,
                           kind="ExternalOutput").ap()

    with tile.TileContext(nc) as tc:
        with (
            tc.tile_pool(name="singles", bufs=1) as singles,
            tc.tile_pool(name="scratch", bufs=1) as scratch,
            tc.tile_pool(name="hps", bufs=1) as hps,
            tc.tile_pool(name="pshid", bufs=2, space="PSUM") as pshid,
            tc.tile_pool(name="pstk", bufs=1, space="PSUM") as pstk,
            tc.tile_pool(name="pstp", bufs=1, space="PSUM") as pstp,
        ):
            # ---- uploads ----
            wcat = singles.tile([128, WCAT_COLS], f16, tag="wcat")
            nc.sync.dma_start(out=wcat, in_=wcat_d)
            bcat = singles.tile([128, BCAT_COLS], f32, tag="bcat")
            nc.sync.dma_start(out=bcat, in_=bcat_d)
            zin = singles.tile([128, NCH, KPQ, C], f32, tag="zin")
            nc.sync.dma_start(out=zin, in_=zin_d)
            z1c = singles.tile([128, STRIP], f32, tag="z1c")
            nc.sync.dma_start(out=z1c, in_=z12_d[0])
            z2c = singles.tile([128, STRIP], f32, tag="z2c")
            nc.sync.dma_start(out=z2c, in_=z12_d[1])

            # ---- x = [zmag(6) ; zr(4)] per 32-partition strip band ----
            xcat = singles.tile([128, STRIP], f16, tag="xcat")
            sq1 = scratch.tile([128, STRIP], f32, tag="sq1")
            sq2 = scratch.tile([128, STRIP], f32, tag="sq2")
            nc.scalar.activation(sq1, z1c, SQUARE)
            nc.scalar.activation(sq2, z2c, SQUARE)
            nc.vector.tensor_add(xcat, sq1, sq2)
            for s in range(NSTRIP):
                nc.sync.dma_start(out=xcat[64 * s + 6:64 * s + 10],
                                  in_=xzr_d[s])

            ident = wcat[:, ID_OFF:ID_OFF + 128]
            bias128 = bcat[:, 15:16]

            # full-width staging for the post phase
            t_all = singles.tile([128, NCH, CHUNK], f16, tag="t_all")
            o_full = singles.tile([128, NCH, KPQ, C], f32, tag="o_full")

            # ReLU engine round-robin: weighted DVE/Act/Pool
            relu_seq = []

            def relu(h, ps, bias_ap):
                i = len(relu_seq) % 2
                relu_seq.append(0)
                if i == 0:
                    nc.vector.tensor_scalar(h, ps, bias_ap, 0.0, ADD, MAX)
                else:
                    nc.scalar.activation(h, ps, RELU, bias=bias_ap)

            # ---- MLP: 4 quarters, weight-stationary phases ----
            # chunk-PAIR granularity: two matmuls of the same pair-block fill
            # one 2-bank PSUM tile; a single [128, 1024] ReLU drains both
            PP = KPQ // 2
            for q in range(NQ):
                s = (q * KPQ) // CPS
                rs = slice(64 * s, 64 * s + 10)
                h0, h1, h2 = {}, {}, {}
                # layer 0
                for j in range(5):
                    w = wcat[rs, W0_OFF + j * 128:W0_OFF + (j + 1) * 128]
                    for pp in range(PP):
                        cc = ((q * KPQ) % CPS + 2 * pp) * CHUNK
                        ps = pshid.tile([128, 2, CHUNK], f32, tag="ps")
                        for u in range(2):
                            nc.tensor.matmul(
                                ps[:, u], w,
                                xcat[rs, cc + u * CHUNK:cc + (u + 1) * CHUNK],
                                start=True, stop=True)
                        h = hps.tile([128, 2, CHUNK], f16, tag=f"h0_{j}_{pp}")
                        relu(h, ps, bcat[:, j:j + 1])
                        h0[j, pp] = h
                # hidden layers
                for l, (off, hin, hout) in enumerate(
                        ((WM0_OFF, h0, h1), (WM1_OFF, h1, h2))):
                    for j in range(5):
                        w = wcat[:, off + j * 128:off + (j + 1) * 128]
                        b = bcat[:, 5 + 5 * l + j:6 + 5 * l + j]
                        for pp in range(PP):
                            ps = pshid.tile([128, 2, CHUNK], f32, tag="ps")
                            for u in range(2):
                                nc.tensor.matmul(ps[:, u], w,
                                                 hin[j, pp][:, u],
                                                 start=True, stop=True)
                            h = hps.tile([128, 2, CHUNK], f16,
                                         tag=f"h{l + 1}_{j}_{pp}")
                            relu(h, ps, b)
                            hout[j, pp] = h
                # final layer: disjoint output rows per j; per chunk-pair so
                # only 2 stk banks stay live
                for pp in range(PP):
                    stks = []
                    for j in range(5):
                        w = wcat[:, WL_OFF + j * 128:WL_OFF + (j + 1) * 128]
                        for u in range(2):
                            if j == 0:
                                stk_t = pstk.tile([128, CHUNK], f32,
                                                  tag=f"stk_{u}")
                                stks.append(stk_t)
                            nc.tensor.matmul(stks[u], w, h2[j, pp][:, u],
                                             start=(j == 0), stop=(j == 4))
                    # post: +bias, fp16, transpose to elem-major
                    for u in range(2):
                        k = q * KPQ + 2 * pp + u
                        sstk = hps.tile([128, CHUNK], f16, tag=f"sstk_{u}")
                        nc.scalar.activation(sstk, stks[u], IDENT,
                                             bias=bias128)
                        tp = pstp.tile([128, CHUNK], f16, tag="tp")
                        for g in range(KPQ):
                            nc.tensor.transpose(
                                tp[:, g * 128:(g + 1) * 128],
                                sstk[:, g * 128:(g + 1) * 128], ident)
                        nc.vector.tensor_copy(t_all[:, k], tp)

            # ---- post: big batched ops over all 16 chunks ----
            # t_all cols within group g: 0-3 lam | 32-37 mu | 64-69 om
            t4 = t_all.rearrange("p k (g c) -> p k g c", g=KPQ, c=128)
            lamT = t4[:, :, :, 0:4]
            muT = t4[:, :, :, 32:38]
            omT = t4[:, :, :, 64:70]

            e_f = singles.tile([128, NCH, KPQ, 6], f32, tag="e_f")
            cs_f = singles.tile([128, NCH, KPQ, 6], f32, tag="cs_f")
            sn_f = singles.tile([128, NCH, KPQ, 6], f32, tag="sn_f")
            nc.scalar.activation(e_f, muT, EXP)
            nc.scalar.activation(cs_f, omT, SIN, bias=bcat[:, 16:17])
            nc.scalar.activation(sn_f, omT, SIN)
            mc_f = singles.tile([128, NCH, KPQ, 6], f32, tag="mc_f")
            ms_f = singles.tile([128, NCH, KPQ, 6], f32, tag="ms_f")
            nc.vector.tensor_mul(mc_f, e_f, cs_f)
            nc.vector.tensor_mul(ms_f, e_f, sn_f)

            zr_v = zin[:, :, :, 0:4]
            z1_v = zin[:, :, :, 4:16:2]
            z2_v = zin[:, :, :, 5:16:2]
            t1f = scratch.tile([128, NCH, KPQ, 6], f32, tag="t1f")
            t2f = scratch.tile([128, NCH, KPQ, 6], f32, tag="t2f")
            nc.vector.tensor_mul(o_full[:, :, :, 0:4], zr_v, lamT)
            nc.vector.tensor_mul(t1f, z1_v, mc_f)
            nc.vector.tensor_mul(t2f, z2_v, ms_f)
            nc.vector.tensor_add(o_full[:, :, :, 4:16:2], t1f, t2f)
            nc.vector.tensor_mul(t1f, z2_v, mc_f)
            nc.vector.tensor_mul(t2f, z1_v, ms_f)
            nc.vector.tensor_sub(o_full[:, :, :, 5:16:2], t1f, t2f)

            nc.sync.dma_start(out=out_d, in_=o_full)

    nc.compile()
    return nc


def _pack_weights(i):
    """Pack per-channel weights into the fused fp16 wcat / fp32 bcat blocks."""
    f32, f16 = np.float32, np.float16
    W0_r, b0_r = np.asarray(i["W0_r"], f32), np.asarray(i["b0_r"], f32)
    Wm_r, bm_r = np.asarray(i["Wm_r"], f32), np.asarray(i["bm_r"], f32)
    Wl_r, bl_r = np.asarray(i["Wl_r"], f32), np.asarray(i["bl_r"], f32)
    W0_c, b0_c = np.asarray(i["W0_c"], f32), np.asarray(i["b0_c"], f32)
    Wm_c, bm_c = np.asarray(i["Wm_c"], f32), np.asarray(i["bm_c"], f32)
    Wl_c, bl_c = np.asarray(i["Wl_c"], f32), np.asarray(i["bl_c"], f32)

    wcat = np.zeros((128, WCAT_COLS), f16)
    bcat = np.zeros((128, BCAT_COLS), f32)
    for j in range(5):
        if j < 2:
            a, b = 2 * j, 2 * j + 1
            W0, b0, Wm, bm = W0_r, b0_r, Wm_r, bm_r
            xra, xrb = 6 + a, 6 + b          # zr rows of x
        else:
            a, b = 2 * (j - 2), 2 * (j - 2) + 1
            W0, b0, Wm, bm = W0_c, b0_c, Wm_c, bm_c
            xra, xrb = a, b                  # zmag rows of x
        # layer 0, replicated at each strip base (partitions 0 and 64)
        for s in range(NSTRIP):
            wcat[64 * s + xra, W0_OFF + j * 128:W0_OFF + j * 128 + 64] = W0[a]
            wcat[64 * s + xrb, W0_OFF + j * 128 + 64:W0_OFF + (j + 1) * 128] \
                = W0[b]
        bcat[0:64, j] = b0[a]
        bcat[64:128, j] = b0[b]
        # hidden layers, block diagonal
        for l, off in enumerate((WM0_OFF, WM1_OFF)):
            wcat[0:64, off + j * 128:off + j * 128 + 64] = Wm[l, a]
            wcat[64:128, off + j * 128 + 64:off + (j + 1) * 128] = Wm[l, b]
            bcat[0:64, 5 + 5 * l + j] = bm[l, a]
            bcat[64:128, 5 + 5 * l + j] = bm[l, b]
        # final layer -> rows 0-3 lam, 32-37 mu, 64-69 om
        wo = WL_OFF + j * 128
        if j < 2:
            wcat[0:64, wo + 2 * j] = Wl_r[a][:, 0]
            wcat[64:128, wo + 2 * j + 1] = Wl_r[b][:, 0]
        else:
            jc = j - 2
            wcat[0:64, wo + 32 + 2 * jc] = Wl_c[a][:, 0]
            wcat[64:128, wo + 33 + 2 * jc] = Wl_c[b][:, 0]
            wcat[0:64, wo + 64 + 2 * jc] = Wl_c[a][:, 1]
            wcat[64:128, wo + 65 + 2 * jc] = Wl_c[b][:, 1]
    wcat[:, ID_OFF:ID_OFF + 128] = np.eye(128, dtype=f16)
    bcat[:, 16] = HALF_PI
    bcat[0:4, 15] = bl_r[:, 0]
    bcat[32:38, 15] = bl_c[:, 0]
    bcat[64:70, 15] = bl_c[:, 1]
    return {"wcat": wcat, "bcat": bcat}


def _pack_z(z_core):
    """Per-core z [8192, 16] -> zin / z12 / xzr DRAM layouts."""
    f32, f16 = np.float32, np.float16
    zc = np.asarray(z_core, f32)
    zin = np.ascontiguousarray(
        zc.reshape(64, 128, C).transpose(1, 0, 2)).reshape(128, NCH, KPQ, C)
    z1 = zc[:, 4:16:2].reshape(NSTRIP, STRIP, 6)   # [s, e, ch]
    z2 = zc[:, 5:16:2].reshape(NSTRIP, STRIP, 6)
    z12 = np.zeros((2, 128, STRIP), f32)
    for s in range(NSTRIP):
        z12[0, 64 * s:64 * s + 6] = z1[s].T
        z12[1, 64 * s:64 * s + 6] = z2[s].T
    xzr = np.ascontiguousarray(
        zc[:, 0:4].reshape(NSTRIP, STRIP, 4).transpose(0, 2, 1)).astype(f16)
    return {"zin": zin, "z12": z12, "xzr": xzr}


def kernel(**inputs):
    global _cached_nc
    if _cached_nc is None:
        _cached_nc = _build()
    nc = _cached_nc

    from concourse.bass_utils import run_bass_kernel_spmd

    weights = _pack_weights(inputs)
    z = np.asarray(inputs["z"], np.float32).reshape(NCORES, F_CORE, C)
    in_maps = [dict(weights, **_pack_z(z[i])) for i in range(NCORES)]
    res = run_bass_kernel_spmd(nc, in_maps, core_ids=list(range(NCORES)))
    outs = [
        np.asarray(res.results[i]["out"])
        .reshape(128, 64, C).transpose(1, 0, 2).reshape(F_CORE, C)
        for i in range(NCORES)
    ]
    return np.concatenate(outs, axis=0).reshape(B, S, C)


# revision 11
# speedup vs baseline: 1.2962x; 1.2962x over previous
"""Trainium2 Bass kernel for the Koopman operator nn.Module.

Per-channel tiny MLPs (4 real channels, 6 complex-conjugate pairs, H=64,
2 hidden layers) over 65536 flattened batch elements, then a block-diagonal
Koopman update.  Pure data parallel over 8 NeuronCores (8192 elements each).

v2 strategy (weight-stationary, fp16 matmul path, transpose-free input):
  - host uploads z in three layouts: elem-major [128, 64, 16] for the final
    combine, strip-packed channel-major z1/z2 (at partition bases 0/32/64/96)
    for the on-device |z|^2, and fp16 zr rows DMA'd straight into the MLP
    input tile -- no input transposes on the tensor engine
  - all matmuls fp16 (1 cycle/row) with fp32 PSUM accumulation
  - 4 quarters x (L0 / hid0 / hid1 / final phases across all 5 pair-blocks):
    consecutive matmuls share stationary weights and the program order lets
    pair j's ReLUs drain while pairs j+1.. stream, keeping the PE p-state
    ramped at 2.4 GHz
  - ReLUs round-robin over DVE / Act / GpSimd (three engines)
  - final-layer outputs go back to elem-major via 4 PE transposes per chunk;
    exp/sin/combine run as a handful of big batched ops at the end
    (sin(x+pi/2) for cos); activation table loads ~2 for the whole kernel
"""

import numpy as np

NR, NCC, L, H = 4, 6, 2, 64
B, S, C = 32, 2048, 16
NCORES = 8
F_CORE = B * S // NCORES        # 8192 elements per core
CHUNK = 512                     # elements per matmul chunk (one PSUM bank)
NCH = F_CORE // CHUNK           # 16 chunks
NQ = 4                          # quarters
KPQ = NCH // NQ                 # 4 chunks per quarter
NSTRIP = 2                      # xcat partition bands at bases 0 and 64
STRIP = F_CORE // NSTRIP        # 4096 elements per strip
CPS = STRIP // CHUNK            # 8 chunks per strip

HALF_PI = float(np.pi / 2)

# wcat column layout (fp16): w0(5x128) | wm0(5x128) | wm1(5x128) | wl(5x128)
# | ident(128)
W0_OFF, WM0_OFF, WM1_OFF, WL_OFF, ID_OFF = 0, 640, 1280, 1920, 2560
WCAT_COLS = 2688
# bcat column layout (fp32): b0(5) | bm0(5) | bm1(5) | bias128(1) | pi/2(1)
BCAT_COLS = 17

_cached_nc = {}


def _build(zero_bias):
    import concourse.tile as tile
    from concourse import bacc, mybir

    f32 = mybir.dt.float32
    f16 = mybir.dt.float16
    RELU = mybir.ActivationFunctionType.Relu
    IDENT = mybir.ActivationFunctionType.Identity
    SQUARE = mybir.ActivationFunctionType.Square
    EXP = mybir.ActivationFunctionType.Exp
    SIN = mybir.ActivationFunctionType.Sin
    ADD = mybir.AluOpType.add
    MAX = mybir.AluOpType.max

    nc = bacc.Bacc("TRN2", target_bir_lowering=False, debug=False,
                   num_devices=NCORES)

    zin_d = nc.dram_tensor("zin", [128, NCH, KPQ, C], f32,
                           kind="ExternalInput").ap()
    z12_d = nc.dram_tensor("z12", [2, 128, STRIP], f32,
                           kind="ExternalInput").ap()
    xzr_d = nc.dram_tensor("xzr", [NSTRIP, 4, STRIP], f16,
                           kind="ExternalInput").ap()
    wcat_d = nc.dram_tensor("wcat", [128, WCAT_COLS], f16,
                            kind="ExternalInput").ap()
    bcat_d = nc.dram_tensor("bcat", [128, BCAT_COLS], f32,
                            kind="ExternalInput").ap()
    out_d = nc.dram_tensor("out", [128, NCH, KPQ, C], f32,
                           kind="ExternalOutput").ap()

    with tile.TileContext(nc) as tc:
        with (
            tc.tile_pool(name="singles", bufs=1) as singles,
            tc.tile_pool(name="scratch", bufs=1) as scratch,
            tc.tile_pool(name="hps", bufs=1) as hps,
            tc.tile_pool(name="pshid", bufs=5, space="PSUM") as pshid,
            tc.tile_pool(name="pstk", bufs=1, space="PSUM") as pstk,
            tc.tile_pool(name="pstp", bufs=1, space="PSUM") as pstp,
        ):
            # ---- uploads ----
            wcat = singles.tile([128, WCAT_COLS], f16, tag="wcat")
            nc.sync.dma_start(out=wcat, in_=wcat_d)
            bcat = singles.tile([128, BCAT_COLS], f32, tag="bcat")
            nc.sync.dma_start(out=bcat, in_=bcat_d)
            zin = singles.tile([128, NCH, KPQ, C], f32, tag="zin")
            nc.sync.dma_start(out=zin, in_=zin_d)
            z1c = singles.tile([128, STRIP], f32, tag="z1c")
            nc.sync.dma_start(out=z1c, in_=z12_d[0])
            z2c = singles.tile([128, STRIP], f32, tag="z2c")
            nc.sync.dma_start(out=z2c, in_=z12_d[1])

            # ---- x = [zmag(6) ; zr(4)] per 32-partition strip band ----
            xcat = singles.tile([128, STRIP], f16, tag="xcat")
            sq1 = scratch.tile([128, STRIP], f32, tag="sq1")
            sq2 = scratch.tile([128, STRIP], f32, tag="sq2")
            nc.scalar.activation(sq1, z1c, SQUARE)
            nc.scalar.activation(sq2, z2c, SQUARE)
            nc.vector.tensor_add(xcat, sq1, sq2)
            for s in range(NSTRIP):
                nc.sync.dma_start(out=xcat[64 * s + 6:64 * s + 10],
                                  in_=xzr_d[s])

            ident = wcat[:, ID_OFF:ID_OFF + 128]
            bias128 = bcat[:, 15:16]

            # full-width staging for the post phase
            t_all = singles.tile([128, NCH, CHUNK], f16, tag="t_all")
            o_full = singles.tile([128, NCH, KPQ, C], f32, tag="o_full")

            # ReLU engine round-robin: weighted DVE/Act/Pool
            relu_seq = []

            def relu(h, ps, bias_ap):
                i = len(relu_seq) % 2
                relu_seq.append(0)
                if i == 0:
                    if zero_bias:
                        nc.vector.tensor_single_scalar(h, ps, 0.0, MAX)
                    else:
                        nc.vector.tensor_scalar(h, ps, bias_ap, 0.0, ADD, MAX)
                else:
                    if zero_bias:
                        nc.scalar.activation(h, ps, RELU)
                    else:
                        nc.scalar.activation(h, ps, RELU, bias=bias_ap)

            # ---- MLP: 4 quarters, weight-stationary phases ----
            for q in range(NQ):
                s = (q * KPQ) // CPS
                rs = slice(64 * s, 64 * s + 10)
                h0, h1, h2 = {}, {}, {}
                # layer 0
                for j in range(5):
                    w = wcat[rs, W0_OFF + j * 128:W0_OFF + (j + 1) * 128]
                    for kk in range(KPQ):
                        cc = ((q * KPQ) % CPS + kk) * CHUNK
                        ps = pshid.tile([128, CHUNK], f32, tag="ps")
                        nc.tensor.matmul(
                            ps, w, xcat[rs, cc:cc + CHUNK],
                            start=True, stop=True)
                        h = hps.tile([128, CHUNK], f16, tag=f"h0_{j}_{kk}")
                        relu(h, ps, bcat[:, j:j + 1])
                        h0[j, kk] = h
                # hidden layers
                for l, (off, hin, hout) in enumerate(
                        ((WM0_OFF, h0, h1), (WM1_OFF, h1, h2))):
                    for j in range(5):
                        w = wcat[:, off + j * 128:off + (j + 1) * 128]
                        b = bcat[:, 5 + 5 * l + j:6 + 5 * l + j]
                        for kk in range(KPQ):
                            ps = pshid.tile([128, CHUNK], f32, tag="ps")
                            nc.tensor.matmul(ps, w, hin[j, kk],
                                             start=True, stop=True)
                            h = hps.tile([128, CHUNK], f16,
                                         tag=f"h{l + 1}_{j}_{kk}")
                            relu(h, ps, b)
                            hout[j, kk] = h
                # final layer: disjoint output rows per j; per chunk-pair so
                # only 2 stk banks stay live
                for pp in range(KPQ // 2):
                    stks = []
                    for j in range(5):
                        w = wcat[:, WL_OFF + j * 128:WL_OFF + (j + 1) * 128]
                        for u in range(2):
                            if j == 0:
                                stk_t = pstk.tile([128, CHUNK], f32,
                                                  tag=f"stk_{u}")
                                stks.append(stk_t)
                            nc.tensor.matmul(stks[u], w, h2[j, 2 * pp + u],
                                             start=(j == 0), stop=(j == 4))
                    # post: +bias, fp16, transpose to elem-major
                    for u in range(2):
                        k = q * KPQ + 2 * pp + u
                        sstk = hps.tile([128, CHUNK], f16, tag=f"sstk_{u}")
                        nc.scalar.activation(sstk, stks[u], IDENT,
                                             bias=bias128)
                        tp = pstp.tile([128, CHUNK], f16, tag="tp")
                        for g in range(KPQ):
                            nc.tensor.transpose(
                                tp[:, g * 128:(g + 1) * 128],
                                sstk[:, g * 128:(g + 1) * 128], ident)
                        nc.vector.tensor_copy(t_all[:, k], tp)

            # ---- post: big batched ops over all 16 chunks ----
            # t_all cols within group g: 0-3 lam | 32-37 mu | 64-69 om
            t4 = t_all.rearrange("p k (g c) -> p k g c", g=KPQ, c=128)
            lamT = t4[:, :, :, 0:4]
            muT = t4[:, :, :, 32:38]
            omT = t4[:, :, :, 64:70]

            e_f = singles.tile([128, NCH, KPQ, 6], f32, tag="e_f")
            cs_f = singles.tile([128, NCH, KPQ, 6], f32, tag="cs_f")
            sn_f = singles.tile([128, NCH, KPQ, 6], f32, tag="sn_f")
            nc.scalar.activation(e_f, muT, EXP)
            nc.scalar.activation(cs_f, omT, SIN, bias=bcat[:, 16:17])
            nc.scalar.activation(sn_f, omT, SIN)
            mc_f = singles.tile([128, NCH, KPQ, 6], f32, tag="mc_f")
            ms_f = singles.tile([128, NCH, KPQ, 6], f32, tag="ms_f")
            nc.vector.tensor_mul(mc_f, e_f, cs_f)
            nc.vector.tensor_mul(ms_f, e_f, sn_f)

            zr_v = zin[:, :, :, 0:4]
            z1_v = zin[:, :, :, 4:16:2]
            z2_v = zin[:, :, :, 5:16:2]
            t1f = scratch.tile([128, NCH, KPQ, 6], f32, tag="t1f")
            t2f = scratch.tile([128, NCH, KPQ, 6], f32, tag="t2f")
            nc.vector.tensor_mul(o_full[:, :, :, 0:4], zr_v, lamT)
            nc.vector.tensor_mul(t1f, z1_v, mc_f)
            nc.vector.tensor_mul(t2f, z2_v, ms_f)
            nc.vector.tensor_add(o_full[:, :, :, 4:16:2], t1f, t2f)
            nc.vector.tensor_mul(t1f, z2_v, mc_f)
            nc.vector.tensor_mul(t2f, z1_v, ms_f)
            nc.vector.tensor_sub(o_full[:, :, :, 5:16:2], t1f, t2f)

            nc.sync.dma_start(out=out_d, in_=o_full)

    nc.compile()
    return nc


def _pack_weights(i):
    """Pack per-channel weights into the fused fp16 wcat / fp32 bcat blocks."""
    f32, f16 = np.float32, np.float16
    W0_r, b0_r = np.asarray(i["W0_r"], f32), np.asarray(i["b0_r"], f32)
    Wm_r, bm_r = np.asarray(i["Wm_r"], f32), np.asarray(i["bm_r"], f32)
    Wl_r, bl_r = np.asarray(i["Wl_r"], f32), np.asarray(i["bl_r"], f32)
    W0_c, b0_c = np.asarray(i["W0_c"], f32), np.asarray(i["b0_c"], f32)
    Wm_c, bm_c = np.asarray(i["Wm_c"], f32), np.asarray(i["bm_c"], f32)
    Wl_c, bl_c = np.asarray(i["Wl_c"], f32), np.asarray(i["bl_c"], f32)

    wcat = np.zeros((128, WCAT_COLS), f16)
    bcat = np.zeros((128, BCAT_COLS), f32)
    for j in range(5):
        if j < 2:
            a, b = 2 * j, 2 * j + 1
            W0, b0, Wm, bm = W0_r, b0_r, Wm_r, bm_r
            xra, xrb = 6 + a, 6 + b          # zr rows of x
        else:
            a, b = 2 * (j - 2), 2 * (j - 2) + 1
            W0, b0, Wm, bm = W0_c, b0_c, Wm_c, bm_c
            xra, xrb = a, b                  # zmag rows of x
        # layer 0, replicated at each strip base (partitions 0 and 64)
        for s in range(NSTRIP):
            wcat[64 * s + xra, W0_OFF + j * 128:W0_OFF + j * 128 + 64] = W0[a]
            wcat[64 * s + xrb, W0_OFF + j * 128 + 64:W0_OFF + (j + 1) * 128] \
                = W0[b]
        bcat[0:64, j] = b0[a]
        bcat[64:128, j] = b0[b]
        # hidden layers, block diagonal
        for l, off in enumerate((WM0_OFF, WM1_OFF)):
            wcat[0:64, off + j * 128:off + j * 128 + 64] = Wm[l, a]
            wcat[64:128, off + j * 128 + 64:off + (j + 1) * 128] = Wm[l, b]
            bcat[0:64, 5 + 5 * l + j] = bm[l, a]
            bcat[64:128, 5 + 5 * l + j] = bm[l, b]
        # final layer -> rows 0-3 lam, 32-37 mu, 64-69 om
        wo = WL_OFF + j * 128
        if j < 2:
            wcat[0:64, wo + 2 * j] = Wl_r[a][:, 0]
            wcat[64:128, wo + 2 * j + 1] = Wl_r[b][:, 0]
        else:
            jc = j - 2
            wcat[0:64, wo + 32 + 2 * jc] = Wl_c[a][:, 0]
            wcat[64:128, wo + 33 + 2 * jc] = Wl_c[b][:, 0]
            wcat[0:64, wo + 64 + 2 * jc] = Wl_c[a][:, 1]
            wcat[64:128, wo + 65 + 2 * jc] = Wl_c[b][:, 1]
    wcat[:, ID_OFF:ID_OFF + 128] = np.eye(128, dtype=f16)
    bcat[:, 16] = HALF_PI
    bcat[0:4, 15] = bl_r[:, 0]
    bcat[32:38, 15] = bl_c[:, 0]
    bcat[64:70, 15] = bl_c[:, 1]
    return {"wcat": wcat, "bcat": bcat}


def _pack_z(z_core):
    """Per-core z [8192, 16] -> zin / z12 / xzr DRAM layouts."""
    f32, f16 = np.float32, np.float16
    zc = np.asarray(z_core, f32)
    zin = np.ascontiguousarray(
        zc.reshape(64, 128, C).transpose(1, 0, 2)).reshape(128, NCH, KPQ, C)
    z1 = zc[:, 4:16:2].reshape(NSTRIP, STRIP, 6)   # [s, e, ch]
    z2 = zc[:, 5:16:2].reshape(NSTRIP, STRIP, 6)
    z12 = np.zeros((2, 128, STRIP), f32)
    for s in range(NSTRIP):
        z12[0, 64 * s:64 * s + 6] = z1[s].T
        z12[1, 64 * s:64 * s + 6] = z2[s].T
    xzr = np.ascontiguousarray(
        zc[:, 0:4].reshape(NSTRIP, STRIP, 4).transpose(0, 2, 1)).astype(f16)
    return {"zin": zin, "z12": z12, "xzr": xzr}


def kernel(**inputs):
    zero_bias = all(
        not np.any(np.asarray(inputs[k]))
        for k in ("b0_r", "bm_r", "bl_r", "b0_c", "bm_c", "bl_c"))
    if zero_bias not in _cached_nc:
        _cached_nc[zero_bias] = _build(zero_bias)
    nc = _cached_nc[zero_bias]

    from concourse.bass_utils import run_bass_kernel_spmd

    weights = _pack_weights(inputs)
    z = np.asarray(inputs["z"], np.float32).reshape(NCORES, F_CORE, C)
    in_maps = [dict(weights, **_pack_z(z[i])) for i in range(NCORES)]
    res = run_bass_kernel_spmd(nc, in_maps, core_ids=list(range(NCORES)))
    outs = [
        np.asarray(res.results[i]["out"])
        .reshape(128, 64, C).transpose(1, 0, 2).reshape(F_CORE, C)
        for i in range(NCORES)
    ]
    return np.concatenate(outs, axis=0).reshape(B, S, C)


# revision 14
# speedup vs baseline: 1.3527x; 1.0436x over previous
"""Trainium2 Bass kernel for the Koopman operator nn.Module.

Per-channel tiny MLPs (4 real channels, 6 complex-conjugate pairs, H=64,
2 hidden layers) over 65536 flattened batch elements, then a block-diagonal
Koopman update.  Pure data parallel over 8 NeuronCores (8192 elements each).

v2 strategy (weight-stationary, fp16 matmul path, transpose-free input):
  - host uploads z in three layouts: elem-major [128, 64, 16] for the final
    combine, strip-packed channel-major z1/z2 (at partition bases 0/32/64/96)
    for the on-device |z|^2, and fp16 zr rows DMA'd straight into the MLP
    input tile -- no input transposes on the tensor engine
  - all matmuls fp16 (1 cycle/row) with fp32 PSUM accumulation
  - 4 quarters x (L0 / hid0 / hid1 / final phases across all 5 pair-blocks):
    consecutive matmuls share stationary weights and the program order lets
    pair j's ReLUs drain while pairs j+1.. stream, keeping the PE p-state
    ramped at 2.4 GHz
  - ReLUs round-robin over DVE / Act / GpSimd (three engines)
  - final-layer outputs go back to elem-major via 4 PE transposes per chunk;
    exp/sin/combine run as a handful of big batched ops at the end
    (sin(x+pi/2) for cos); activation table loads ~2 for the whole kernel
"""

import numpy as np

NR, NCC, L, H = 4, 6, 2, 64
B, S, C = 32, 2048, 16
NCORES = 8
F_CORE = B * S // NCORES        # 8192 elements per core
CHUNK = 512                     # elements per matmul chunk (one PSUM bank)
NCH = F_CORE // CHUNK           # 16 chunks
NQ = 4                          # quarters
KPQ = NCH // NQ                 # 4 chunks per quarter
NSTRIP = 2                      # xcat partition bands at bases 0 and 64
STRIP = F_CORE // NSTRIP        # 4096 elements per strip
CPS = STRIP // CHUNK            # 8 chunks per strip

HALF_PI = float(np.pi / 2)

# wcat column layout (fp16): w0(5x128) | wm0(5x128) | wm1(5x128) | wl(5x128)
# | ident(128)
W0_OFF, WM0_OFF, WM1_OFF, WL_OFF, ID_OFF = 0, 640, 1280, 1920, 2560
WCAT_COLS = 2688
# bcat column layout (fp32): b0(5) | bm0(5) | bm1(5) | bias128(1) | pi/2(1)
BCAT_COLS = 17

_cached_nc = {}


def _build(zero_bias):
    import concourse.tile as tile
    from concourse import bacc, mybir

    f32 = mybir.dt.float32
    f16 = mybir.dt.float16
    RELU = mybir.ActivationFunctionType.Relu
    IDENT = mybir.ActivationFunctionType.Identity
    SQUARE = mybir.ActivationFunctionType.Square
    EXP = mybir.ActivationFunctionType.Exp
    SIN = mybir.ActivationFunctionType.Sin
    ADD = mybir.AluOpType.add
    MAX = mybir.AluOpType.max

    nc = bacc.Bacc("TRN2", target_bir_lowering=False, debug=False,
                   num_devices=NCORES)

    zin_d = nc.dram_tensor("zin", [128, NCH, KPQ, C], f32,
                           kind="ExternalInput").ap()
    wcat_d = nc.dram_tensor("wcat", [128, WCAT_COLS], f16,
                            kind="ExternalInput").ap()
    bcat_d = nc.dram_tensor("bcat", [128, BCAT_COLS], f32,
                            kind="ExternalInput").ap()
    out_d = nc.dram_tensor("out", [128, NCH, KPQ, C], f32,
                           kind="ExternalOutput").ap()

    with tile.TileContext(nc) as tc:
        with (
            tc.tile_pool(name="singles", bufs=1) as singles,
            tc.tile_pool(name="scratch", bufs=1) as scratch,
            tc.tile_pool(name="hps", bufs=1) as hps,
            tc.tile_pool(name="pshid", bufs=4, space="PSUM") as pshid,
            tc.tile_pool(name="pstk", bufs=1, space="PSUM") as pstk,
            tc.tile_pool(name="pstp", bufs=1, space="PSUM") as pstp,
        ):
            # ---- uploads ----
            wcat = singles.tile([128, WCAT_COLS], f16, tag="wcat")
            nc.sync.dma_start(out=wcat, in_=wcat_d)
            bcat = singles.tile([128, BCAT_COLS], f32, tag="bcat")
            nc.sync.dma_start(out=bcat, in_=bcat_d)
            zin = singles.tile([128, NCH, KPQ, C], f32, tag="zin")
            nc.sync.dma_start(out=zin, in_=zin_d)

            # ---- x = [zmag(6) ; zr(4)]: build elem-major, PE-transpose to
            # channel-major per chunk, DMA PSUM->SBUF into the strip bands
            xcat = singles.tile([74, 6 * CHUNK], f16, tag="xcat")
            x_nat = singles.tile([128, NCH, KPQ, 10], f16, tag="x_nat")
            sq1 = scratch.tile([128, NCH, KPQ, 6], f32, tag="sq1")
            sq2 = scratch.tile([128, NCH, KPQ, 6], f32, tag="sq2")
            z1_vv = zin[:, :, :, 4:16:2]
            z2_vv = zin[:, :, :, 5:16:2]
            nc.vector.tensor_mul(sq1, z1_vv, z1_vv)
            nc.gpsimd.tensor_mul(sq2, z2_vv, z2_vv)
            nc.vector.tensor_add(x_nat[:, :, :, 0:6], sq1, sq2)
            nc.vector.tensor_copy(x_nat[:, :, :, 6:10], zin[:, :, :, 0:4])

            ident = wcat[:, ID_OFF:ID_OFF + 128]
            bias128 = bcat[:, 15:16]

            # full-width staging for the post phase
            t_all = singles.tile([128, NCH, CHUNK], f16, tag="t_all")
            o_full = singles.tile([128, NCH, KPQ, C], f32, tag="o_full")

            # ReLU engine round-robin: weighted DVE/Act/Pool
            relu_seq = []

            def relu(h, ps, bias_ap):
                i = len(relu_seq) % 2
                relu_seq.append(0)
                if i == 0:
                    if zero_bias:
                        nc.vector.tensor_single_scalar(h, ps, 0.0, MAX)
                    else:
                        nc.vector.tensor_scalar(h, ps, bias_ap, 0.0, ADD, MAX)
                else:
                    if zero_bias:
                        nc.scalar.activation(h, ps, RELU)
                    else:
                        nc.scalar.activation(h, ps, RELU, bias=bias_ap)

            psx_done = set()

            def emit_psx(g3):
                # transpose up to 3 chunks of x_nat into channel-major rows
                # stacked at partition bases 0/32/64 of one PSUM bank, then
                # one engine copy moves all of them into xcat
                if g3 in psx_done:
                    return
                psx_done.add(g3)
                ks = [k for k in (3 * g3, 3 * g3 + 1, 3 * g3 + 2) if k < NCH]
                rows = 32 * (len(ks) - 1) + 10
                psx = pstp.tile([74, CHUNK], f16, tag="psx")
                for i, k in enumerate(ks):
                    for g in range(KPQ):
                        nc.tensor.transpose(
                            psx[32 * i:32 * i + 10, g * 128:(g + 1) * 128],
                            x_nat[:, k, g], ident)
                nc.vector.tensor_copy(
                    xcat[0:rows, g3 * CHUNK:(g3 + 1) * CHUNK],
                    psx[0:rows])

            # ---- MLP: 4 quarters, weight-stationary phases ----
            for q in range(NQ):
                h0, h1, h2 = {}, {}, {}
                # layer 0 (chunk-outer so quarter 0 streams as soon as the
                # first chunks of x are transposed; prefetch next quarter's x)
                for kk in range(KPQ):
                    k = q * KPQ + kk
                    if q == 0:
                        emit_psx(k // 3)
                    m, cg = k % 3, k // 3
                    rsk = slice(32 * m, 32 * m + 10)
                    cc = cg * CHUNK
                    for j in range(5):
                        w = wcat[rsk, W0_OFF + j * 128:W0_OFF + (j + 1) * 128]
                        ps = pshid.tile([128, CHUNK], f32, tag="ps")
                        nc.tensor.matmul(
                            ps, w, xcat[rsk, cc:cc + CHUNK],
                            start=True, stop=True)
                        h = hps.tile([128, CHUNK], f16, tag=f"h0_{j}_{kk}")
                        relu(h, ps, bcat[:, j:j + 1])
                        h0[j, kk] = h
                if q < NQ - 1:
                    for kn in range((q + 1) * KPQ, (q + 2) * KPQ):
                        emit_psx(kn // 3)
                # hidden layers
                for l, (off, hin, hout) in enumerate(
                        ((WM0_OFF, h0, h1), (WM1_OFF, h1, h2))):
                    for j in range(5):
                        w = wcat[:, off + j * 128:off + (j + 1) * 128]
                        b = bcat[:, 5 + 5 * l + j:6 + 5 * l + j]
                        for kk in range(KPQ):
                            ps = pshid.tile([128, CHUNK], f32, tag="ps")
                            nc.tensor.matmul(ps, w, hin[j, kk],
                                             start=True, stop=True)
                            h = hps.tile([128, CHUNK], f16,
                                         tag=f"h{l + 1}_{j}_{kk}")
                            relu(h, ps, b)
                            hout[j, kk] = h
                # final layer: disjoint output rows per j; per chunk-pair so
                # only 2 stk banks stay live
                for pp in range(KPQ // 2):
                    stks = []
                    for j in range(5):
                        w = wcat[:, WL_OFF + j * 128:WL_OFF + (j + 1) * 128]
                        for u in range(2):
                            if j == 0:
                                stk_t = pstk.tile([128, CHUNK], f32,
                                                  tag=f"stk_{u}")
                                stks.append(stk_t)
                            nc.tensor.matmul(stks[u], w, h2[j, 2 * pp + u],
                                             start=(j == 0), stop=(j == 4))
                    # post: +bias, fp16, transpose to elem-major
                    for u in range(2):
                        k = q * KPQ + 2 * pp + u
                        sstk = hps.tile([128, CHUNK], f16, tag=f"sstk_{u}")
                        nc.scalar.activation(sstk, stks[u], IDENT,
                                             bias=bias128)
                        tp = pstp.tile([128, CHUNK], f16, tag="tp")
                        for g in range(KPQ):
                            nc.tensor.transpose(
                                tp[:, g * 128:(g + 1) * 128],
                                sstk[:, g * 128:(g + 1) * 128], ident)
                        nc.vector.tensor_copy(t_all[:, k], tp)

                # half-post: exp/sin + combine for chunks [0,8) after q=1,
                # [8,16) after q=3; overlaps the next quarters' MLP
                if q in (1, 3):
                    hs = slice((q // 2) * 8, (q // 2) * 8 + 8)
                    t4 = t_all.rearrange("p k (g c) -> p k g c",
                                         g=KPQ, c=128)
                    lamT = t4[:, hs, :, 0:4]
                    muT = t4[:, hs, :, 32:38]
                    omT = t4[:, hs, :, 64:70]
                    e_f = scratch.tile([128, 8, KPQ, 6], f32, tag="e_f")
                    cs_f = scratch.tile([128, 8, KPQ, 6], f32, tag="cs_f")
                    sn_f = scratch.tile([128, 8, KPQ, 6], f32, tag="sn_f")
                    nc.scalar.activation(e_f, muT, EXP)
                    nc.scalar.activation(cs_f, omT, SIN, bias=bcat[:, 16:17])
                    nc.scalar.activation(sn_f, omT, SIN)
                    mc_f = scratch.tile([128, 8, KPQ, 6], f32, tag="mc_f")
                    ms_f = scratch.tile([128, 8, KPQ, 6], f32, tag="ms_f")
                    nc.vector.tensor_mul(mc_f, e_f, cs_f)
                    nc.vector.tensor_mul(ms_f, e_f, sn_f)
                    zr_v = zin[:, hs, :, 0:4]
                    z1_v = zin[:, hs, :, 4:16:2]
                    z2_v = zin[:, hs, :, 5:16:2]
                    ov = o_full[:, hs]
                    t1f = scratch.tile([128, 8, KPQ, 6], f32, tag="t1f")
                    t2f = scratch.tile([128, 8, KPQ, 6], f32, tag="t2f")
                    nc.vector.tensor_mul(ov[:, :, :, 0:4], zr_v, lamT)
                    nc.vector.tensor_mul(t1f, z1_v, mc_f)
                    nc.vector.tensor_mul(t2f, z2_v, ms_f)
                    nc.vector.tensor_add(ov[:, :, :, 4:16:2], t1f, t2f)
                    nc.vector.tensor_mul(t1f, z2_v, mc_f)
                    nc.vector.tensor_mul(t2f, z1_v, ms_f)
                    nc.vector.tensor_sub(ov[:, :, :, 5:16:2], t1f, t2f)
                    nc.sync.dma_start(out=out_d[:, hs], in_=ov)



    nc.compile()
    return nc


def _pack_weights(i):
    """Pack per-channel weights into the fused fp16 wcat / fp32 bcat blocks."""
    f32, f16 = np.float32, np.float16
    W0_r, b0_r = np.asarray(i["W0_r"], f32), np.asarray(i["b0_r"], f32)
    Wm_r, bm_r = np.asarray(i["Wm_r"], f32), np.asarray(i["bm_r"], f32)
    Wl_r, bl_r = np.asarray(i["Wl_r"], f32), np.asarray(i["bl_r"], f32)
    W0_c, b0_c = np.asarray(i["W0_c"], f32), np.asarray(i["b0_c"], f32)
    Wm_c, bm_c = np.asarray(i["Wm_c"], f32), np.asarray(i["bm_c"], f32)
    Wl_c, bl_c = np.asarray(i["Wl_c"], f32), np.asarray(i["bl_c"], f32)

    wcat = np.zeros((128, WCAT_COLS), f16)
    bcat = np.zeros((128, BCAT_COLS), f32)
    for j in range(5):
        if j < 2:
            a, b = 2 * j, 2 * j + 1
            W0, b0, Wm, bm = W0_r, b0_r, Wm_r, bm_r
            xra, xrb = 6 + a, 6 + b          # zr rows of x
        else:
            a, b = 2 * (j - 2), 2 * (j - 2) + 1
            W0, b0, Wm, bm = W0_c, b0_c, Wm_c, bm_c
            xra, xrb = a, b                  # zmag rows of x
        # layer 0, replicated at partition bases 0/32/64
        for m in range(3):
            wcat[32 * m + xra, W0_OFF + j * 128:W0_OFF + j * 128 + 64] = W0[a]
            wcat[32 * m + xrb, W0_OFF + j * 128 + 64:W0_OFF + (j + 1) * 128] \
                = W0[b]
        bcat[0:64, j] = b0[a]
        bcat[64:128, j] = b0[b]
        # hidden layers, block diagonal
        for l, off in enumerate((WM0_OFF, WM1_OFF)):
            wcat[0:64, off + j * 128:off + j * 128 + 64] = Wm[l, a]
            wcat[64:128, off + j * 128 + 64:off + (j + 1) * 128] = Wm[l, b]
            bcat[0:64, 5 + 5 * l + j] = bm[l, a]
            bcat[64:128, 5 + 5 * l + j] = bm[l, b]
        # final layer -> rows 0-3 lam, 32-37 mu, 64-69 om
        wo = WL_OFF + j * 128
        if j < 2:
            wcat[0:64, wo + 2 * j] = Wl_r[a][:, 0]
            wcat[64:128, wo + 2 * j + 1] = Wl_r[b][:, 0]
        else:
            jc = j - 2
            wcat[0:64, wo + 32 + 2 * jc] = Wl_c[a][:, 0]
            wcat[64:128, wo + 33 + 2 * jc] = Wl_c[b][:, 0]
            wcat[0:64, wo + 64 + 2 * jc] = Wl_c[a][:, 1]
            wcat[64:128, wo + 65 + 2 * jc] = Wl_c[b][:, 1]
    wcat[:, ID_OFF:ID_OFF + 128] = np.eye(128, dtype=f16)
    bcat[:, 16] = HALF_PI
    bcat[0:4, 15] = bl_r[:, 0]
    bcat[32:38, 15] = bl_c[:, 0]
    bcat[64:70, 15] = bl_c[:, 1]
    return {"wcat": wcat, "bcat": bcat}


def _pack_z(z_core):
    """Per-core z [8192, 16] -> elem-major zin DRAM layout."""
    zc = np.asarray(z_core, np.float32)
    zin = np.ascontiguousarray(
        zc.reshape(64, 128, C).transpose(1, 0, 2)).reshape(128, NCH, KPQ, C)
    return {"zin": zin}


def kernel(**inputs):
    zero_bias = all(
        not np.any(np.asarray(inputs[k]))
        for k in ("b0_r", "bm_r", "bl_r", "b0_c", "bm_c", "bl_c"))
    if zero_bias not in _cached_nc:
        _cached_nc[zero_bias] = _build(zero_bias)
    nc = _cached_nc[zero_bias]

    from concourse.bass_utils import run_bass_kernel_spmd

    weights = _pack_weights(inputs)
    z = np.asarray(inputs["z"], np.float32).reshape(NCORES, F_CORE, C)
    in_maps = [dict(weights, **_pack_z(z[i])) for i in range(NCORES)]
    res = run_bass_kernel_spmd(nc, in_maps, core_ids=list(range(NCORES)))
    outs = [
        np.asarray(res.results[i]["out"])
        .reshape(128, 64, C).transpose(1, 0, 2).reshape(F_CORE, C)
        for i in range(NCORES)
    ]
    return np.concatenate(outs, axis=0).reshape(B, S, C)


# revision 15
# speedup vs baseline: 1.4335x; 1.0597x over previous
"""Trainium2 Bass kernel for the Koopman operator nn.Module.

Per-channel tiny MLPs (4 real channels, 6 complex-conjugate pairs, H=64,
2 hidden layers) over 65536 flattened batch elements, then a block-diagonal
Koopman update.  Pure data parallel over 8 NeuronCores (8192 elements each).

v2 strategy (weight-stationary, fp16 matmul path, transpose-free input):
  - host uploads z in three layouts: elem-major [128, 64, 16] for the final
    combine, strip-packed channel-major z1/z2 (at partition bases 0/32/64/96)
    for the on-device |z|^2, and fp16 zr rows DMA'd straight into the MLP
    input tile -- no input transposes on the tensor engine
  - all matmuls fp16 (1 cycle/row) with fp32 PSUM accumulation
  - 4 quarters x (L0 / hid0 / hid1 / final phases across all 5 pair-blocks):
    consecutive matmuls share stationary weights and the program order lets
    pair j's ReLUs drain while pairs j+1.. stream, keeping the PE p-state
    ramped at 2.4 GHz
  - ReLUs round-robin over DVE / Act / GpSimd (three engines)
  - final-layer outputs go back to elem-major via 4 PE transposes per chunk;
    exp/sin/combine run as a handful of big batched ops at the end
    (sin(x+pi/2) for cos); activation table loads ~2 for the whole kernel
"""

import numpy as np

NR, NCC, L, H = 4, 6, 2, 64
B, S, C = 32, 2048, 16
NCORES = 8
F_CORE = B * S // NCORES        # 8192 elements per core
CHUNK = 512                     # elements per matmul chunk (one PSUM bank)
NCH = F_CORE // CHUNK           # 16 chunks
NQ = 4                          # quarters
KPQ = NCH // NQ                 # 4 chunks per quarter
NSTRIP = 2                      # xcat partition bands at bases 0 and 64
STRIP = F_CORE // NSTRIP        # 4096 elements per strip
CPS = STRIP // CHUNK            # 8 chunks per strip

HALF_PI = float(np.pi / 2)

# wcat column layout (fp16): w0(5x128) | wm0(5x128) | wm1(5x128) | wl(5x128)
# | ident(128)
W0_OFF, WM0_OFF, WM1_OFF, WL_OFF, ID_OFF = 0, 640, 1280, 1920, 2560
WCAT_COLS = 2688
# bcat column layout (fp32): b0(5) | bm0(5) | bm1(5) | bias128(1) | pi/2(1)
BCAT_COLS = 17

_cached_nc = {}


def _build(zero_bias):
    import concourse.tile as tile
    from concourse import bacc, mybir

    f32 = mybir.dt.float32
    f16 = mybir.dt.float16
    RELU = mybir.ActivationFunctionType.Relu
    IDENT = mybir.ActivationFunctionType.Identity
    SQUARE = mybir.ActivationFunctionType.Square
    EXP = mybir.ActivationFunctionType.Exp
    SIN = mybir.ActivationFunctionType.Sin
    ADD = mybir.AluOpType.add
    MAX = mybir.AluOpType.max

    nc = bacc.Bacc("TRN2", target_bir_lowering=False, debug=False,
                   num_devices=NCORES)

    zin_d = nc.dram_tensor("zin", [128, NCH, KPQ, C], f32,
                           kind="ExternalInput").ap()
    wcat_d = nc.dram_tensor("wcat", [128, WCAT_COLS], f16,
                            kind="ExternalInput").ap()
    bcat_d = nc.dram_tensor("bcat", [128, BCAT_COLS], f32,
                            kind="ExternalInput").ap()
    out_d = nc.dram_tensor("out", [128, NCH, KPQ, C], f32,
                           kind="ExternalOutput").ap()

    with tile.TileContext(nc) as tc:
        with (
            tc.tile_pool(name="singles", bufs=1) as singles,
            tc.tile_pool(name="scratch", bufs=1) as scratch,
            tc.tile_pool(name="hps", bufs=1) as hps,
            tc.tile_pool(name="pshid", bufs=5, space="PSUM") as pshid,
            tc.tile_pool(name="pstk", bufs=1, space="PSUM") as pstk,
            tc.tile_pool(name="pstp", bufs=1, space="PSUM") as pstp,
        ):
            # ---- uploads ----
            wcat = singles.tile([128, WCAT_COLS], f16, tag="wcat")
            nc.sync.dma_start(out=wcat, in_=wcat_d)
            bcat = singles.tile([128, BCAT_COLS], f32, tag="bcat")
            nc.sync.dma_start(out=bcat, in_=bcat_d)
            zin = singles.tile([128, NCH, KPQ, C], f32, tag="zin")

            # ---- x = [zmag(6) ; zr(4)]: build elem-major per quarter,
            # PE-transpose pairs of chunks to partition bases 0/32, one
            # engine copy into xcat per pair
            xcat = singles.tile([42, NCH * CHUNK // 2], f16, tag="xcat")
            x_nat = singles.tile([128, NCH, KPQ, 10], f16, tag="x_nat")

            def emit_xnat(qx):
                qs = slice(qx * KPQ, (qx + 1) * KPQ)
                nc.sync.dma_start(out=zin[:, qs], in_=zin_d[:, qs])
                sq1 = scratch.tile([128, KPQ, KPQ, 6], f32, tag="sq1")
                sq2 = scratch.tile([128, KPQ, KPQ, 6], f32, tag="sq2")
                z1_vv = zin[:, qs, :, 4:16:2]
                z2_vv = zin[:, qs, :, 5:16:2]
                nc.vector.tensor_mul(sq1, z1_vv, z1_vv)
                nc.gpsimd.tensor_mul(sq2, z2_vv, z2_vv)
                nc.vector.tensor_add(x_nat[:, qs, :, 0:6], sq1, sq2)
                nc.vector.tensor_copy(x_nat[:, qs, :, 6:10],
                                      zin[:, qs, :, 0:4])

            ident = wcat[:, ID_OFF:ID_OFF + 128]
            bias128 = bcat[:, 15:16]

            # full-width staging for the post phase
            t_all = singles.tile([128, NCH, CHUNK], f16, tag="t_all")
            o_full = singles.tile([128, NCH, KPQ, C], f32, tag="o_full")

            # ReLU engine round-robin: weighted DVE/Act/Pool
            relu_seq = []

            def relu(h, ps, bias_ap):
                i = len(relu_seq) % 2
                relu_seq.append(0)
                if i == 0:
                    if zero_bias:
                        nc.vector.tensor_single_scalar(h, ps, 0.0, MAX)
                    else:
                        nc.vector.tensor_scalar(h, ps, bias_ap, 0.0, ADD, MAX)
                else:
                    if zero_bias:
                        nc.scalar.activation(h, ps, RELU)
                    else:
                        nc.scalar.activation(h, ps, RELU, bias=bias_ap)

            def emit_psx(g2):
                # transpose a pair of chunks of x_nat into channel-major rows
                # stacked at partition bases 0/32 of one PSUM bank, then one
                # engine copy moves both into xcat
                psx = pstp.tile([128, CHUNK], f16, tag="tpx")
                for i, k in enumerate((2 * g2, 2 * g2 + 1)):
                    for g in range(KPQ):
                        nc.tensor.transpose(
                            psx[32 * i:32 * i + 10, g * 128:(g + 1) * 128],
                            x_nat[:, k, g], ident)
                nc.vector.tensor_copy(
                    xcat[:, g2 * CHUNK:(g2 + 1) * CHUNK], psx[0:42])

            # ---- MLP: 4 quarters, weight-stationary phases ----
            emit_xnat(0)
            for q in range(NQ):
                h0, h1, h2 = {}, {}, {}
                # layer 0 (chunk-outer so quarter 0 streams as soon as the
                # first chunks of x are transposed; prefetch next quarter's x)
                for kk in range(KPQ):
                    k = q * KPQ + kk
                    if q == 0 and kk % 2 == 0:
                        emit_psx(k // 2)
                    m, cg = k % 2, k // 2
                    rsk = slice(32 * m, 32 * m + 10)
                    cc = cg * CHUNK
                    for j in range(5):
                        w = wcat[rsk, W0_OFF + j * 128:W0_OFF + (j + 1) * 128]
                        ps = pshid.tile([128, CHUNK], f32, tag="ps")
                        nc.tensor.matmul(
                            ps, w, xcat[rsk, cc:cc + CHUNK],
                            start=True, stop=True)
                        h = hps.tile([128, CHUNK], f16, tag=f"h0_{j}_{kk}")
                        relu(h, ps, bcat[:, j:j + 1])
                        h0[j, kk] = h
                if q < NQ - 1:
                    emit_xnat(q + 1)
                    emit_psx(2 * (q + 1))
                    emit_psx(2 * (q + 1) + 1)
                # hidden layers
                for l, (off, hin, hout) in enumerate(
                        ((WM0_OFF, h0, h1), (WM1_OFF, h1, h2))):
                    for j in range(5):
                        w = wcat[:, off + j * 128:off + (j + 1) * 128]
                        b = bcat[:, 5 + 5 * l + j:6 + 5 * l + j]
                        for kk in range(KPQ):
                            ps = pshid.tile([128, CHUNK], f32, tag="ps")
                            nc.tensor.matmul(ps, w, hin[j, kk],
                                             start=True, stop=True)
                            h = hps.tile([128, CHUNK], f16,
                                         tag=f"h{l + 1}_{j}_{kk}")
                            relu(h, ps, b)
                            hout[j, kk] = h
                # final layer: disjoint output rows per j; per chunk-pair so
                # only 2 stk banks stay live
                for pp in range(KPQ // 2):
                    stks = []
                    for j in range(5):
                        w = wcat[:, WL_OFF + j * 128:WL_OFF + (j + 1) * 128]
                        for u in range(2):
                            if j == 0:
                                stk_t = pstk.tile([128, CHUNK], f32,
                                                  tag=f"stk_{u}")
                                stks.append(stk_t)
                            nc.tensor.matmul(stks[u], w, h2[j, 2 * pp + u],
                                             start=(j == 0), stop=(j == 4))
                    # post: +bias, fp16, transpose to elem-major
                    for u in range(2):
                        k = q * KPQ + 2 * pp + u
                        sstk = hps.tile([128, CHUNK], f16, tag=f"sstk_{u}")
                        nc.scalar.activation(sstk, stks[u], IDENT,
                                             bias=bias128)
                        tp = pstp.tile([128, CHUNK], f16, tag="tpx")
                        for g in range(KPQ):
                            nc.tensor.transpose(
                                tp[:, g * 128:(g + 1) * 128],
                                sstk[:, g * 128:(g + 1) * 128], ident)
                        nc.vector.tensor_copy(t_all[:, k], tp)

                # post batches: chunks [0,8) after q=1, [8,12) after q=2,
                # [12,16) after q=3; overlaps the next quarters' MLP
                if q >= 1:
                    hs = slice(0, 8) if q == 1 else slice(q * KPQ, (q + 1) * KPQ)
                    nb = hs.stop - hs.start
                    t4 = t_all.rearrange("p k (g c) -> p k g c",
                                         g=KPQ, c=128)
                    lamT = t4[:, hs, :, 0:4]
                    muT = t4[:, hs, :, 32:38]
                    omT = t4[:, hs, :, 64:70]
                    e_f = scratch.tile([128, nb, KPQ, 6], f32, tag="e_f")
                    cs_f = scratch.tile([128, nb, KPQ, 6], f32, tag="cs_f")
                    sn_f = scratch.tile([128, nb, KPQ, 6], f32, tag="sn_f")
                    nc.scalar.activation(e_f, muT, EXP)
                    nc.scalar.activation(cs_f, omT, SIN, bias=bcat[:, 16:17])
                    nc.scalar.activation(sn_f, omT, SIN)
                    mc_f = scratch.tile([128, nb, KPQ, 6], f32, tag="mc_f")
                    ms_f = scratch.tile([128, nb, KPQ, 6], f32, tag="ms_f")
                    nc.vector.tensor_mul(mc_f, e_f, cs_f)
                    nc.vector.tensor_mul(ms_f, e_f, sn_f)
                    zr_v = zin[:, hs, :, 0:4]
                    z1_v = zin[:, hs, :, 4:16:2]
                    z2_v = zin[:, hs, :, 5:16:2]
                    ov = o_full[:, hs]
                    t1f = scratch.tile([128, nb, KPQ, 6], f32, tag="t1f")
                    t2f = scratch.tile([128, nb, KPQ, 6], f32, tag="t2f")
                    nc.vector.tensor_mul(ov[:, :, :, 0:4], zr_v, lamT)
                    nc.vector.tensor_mul(t1f, z1_v, mc_f)
                    nc.vector.tensor_mul(t2f, z2_v, ms_f)
                    nc.vector.tensor_add(ov[:, :, :, 4:16:2], t1f, t2f)
                    nc.vector.tensor_mul(t1f, z2_v, mc_f)
                    nc.vector.tensor_mul(t2f, z1_v, ms_f)
                    nc.vector.tensor_sub(ov[:, :, :, 5:16:2], t1f, t2f)
                    nc.sync.dma_start(out=out_d[:, hs], in_=ov)



    nc.compile()
    return nc


def _pack_weights(i):
    """Pack per-channel weights into the fused fp16 wcat / fp32 bcat blocks."""
    f32, f16 = np.float32, np.float16
    W0_r, b0_r = np.asarray(i["W0_r"], f32), np.asarray(i["b0_r"], f32)
    Wm_r, bm_r = np.asarray(i["Wm_r"], f32), np.asarray(i["bm_r"], f32)
    Wl_r, bl_r = np.asarray(i["Wl_r"], f32), np.asarray(i["bl_r"], f32)
    W0_c, b0_c = np.asarray(i["W0_c"], f32), np.asarray(i["b0_c"], f32)
    Wm_c, bm_c = np.asarray(i["Wm_c"], f32), np.asarray(i["bm_c"], f32)
    Wl_c, bl_c = np.asarray(i["Wl_c"], f32), np.asarray(i["bl_c"], f32)

    wcat = np.zeros((128, WCAT_COLS), f16)
    bcat = np.zeros((128, BCAT_COLS), f32)
    for j in range(5):
        if j < 2:
            a, b = 2 * j, 2 * j + 1
            W0, b0, Wm, bm = W0_r, b0_r, Wm_r, bm_r
            xra, xrb = 6 + a, 6 + b          # zr rows of x
        else:
            a, b = 2 * (j - 2), 2 * (j - 2) + 1
            W0, b0, Wm, bm = W0_c, b0_c, Wm_c, bm_c
            xra, xrb = a, b                  # zmag rows of x
        # layer 0, replicated at partition bases 0/32
        for m in range(2):
            wcat[32 * m + xra, W0_OFF + j * 128:W0_OFF + j * 128 + 64] = W0[a]
            wcat[32 * m + xrb, W0_OFF + j * 128 + 64:W0_OFF + (j + 1) * 128] \
                = W0[b]
        bcat[0:64, j] = b0[a]
        bcat[64:128, j] = b0[b]
        # hidden layers, block diagonal
        for l, off in enumerate((WM0_OFF, WM1_OFF)):
            wcat[0:64, off + j * 128:off + j * 128 + 64] = Wm[l, a]
            wcat[64:128, off + j * 128 + 64:off + (j + 1) * 128] = Wm[l, b]
            bcat[0:64, 5 + 5 * l + j] = bm[l, a]
            bcat[64:128, 5 + 5 * l + j] = bm[l, b]
        # final layer -> rows 0-3 lam, 32-37 mu, 64-69 om
        wo = WL_OFF + j * 128
        if j < 2:
            wcat[0:64, wo + 2 * j] = Wl_r[a][:, 0]
            wcat[64:128, wo + 2 * j + 1] = Wl_r[b][:, 0]
        else:
            jc = j - 2
            wcat[0:64, wo + 32 + 2 * jc] = Wl_c[a][:, 0]
            wcat[64:128, wo + 33 + 2 * jc] = Wl_c[b][:, 0]
            wcat[0:64, wo + 64 + 2 * jc] = Wl_c[a][:, 1]
            wcat[64:128, wo + 65 + 2 * jc] = Wl_c[b][:, 1]
    wcat[:, ID_OFF:ID_OFF + 128] = np.eye(128, dtype=f16)
    bcat[:, 16] = HALF_PI
    bcat[0:4, 15] = bl_r[:, 0]
    bcat[32:38, 15] = bl_c[:, 0]
    bcat[64:70, 15] = bl_c[:, 1]
    return {"wcat": wcat, "bcat": bcat}


def _pack_z(z_core):
    """Per-core z [8192, 16] -> elem-major zin DRAM layout."""
    zc = np.asarray(z_core, np.float32)
    zin = np.ascontiguousarray(
        zc.reshape(64, 128, C).transpose(1, 0, 2)).reshape(128, NCH, KPQ, C)
    return {"zin": zin}


def kernel(**inputs):
    zero_bias = all(
        not np.any(np.asarray(inputs[k]))
        for k in ("b0_r", "bm_r", "bl_r", "b0_c", "bm_c", "bl_c"))
    if zero_bias not in _cached_nc:
        _cached_nc[zero_bias] = _build(zero_bias)
    nc = _cached_nc[zero_bias]

    from concourse.bass_utils import run_bass_kernel_spmd

    weights = _pack_weights(inputs)
    z = np.asarray(inputs["z"], np.float32).reshape(NCORES, F_CORE, C)
    in_maps = [dict(weights, **_pack_z(z[i])) for i in range(NCORES)]
    res = run_bass_kernel_spmd(nc, in_maps, core_ids=list(range(NCORES)))
    outs = [
        np.asarray(res.results[i]["out"])
        .reshape(128, 64, C).transpose(1, 0, 2).reshape(F_CORE, C)
        for i in range(NCORES)
    ]
    return np.concatenate(outs, axis=0).reshape(B, S, C)
